# revision 6
# baseline (speedup 1.0000x reference)
"""Causal self-attention (B=2, T=4096, C=768, H=12) on 8 trn2 NeuronCores.

Sharding: core c -> (batch b = c//4, head-group g = c%4, heads [3g, 3g+1, 3g+2]).
Each core computes q/k/v + causal attention + proj-partial for its 3 heads of
its batch, then a ReduceScatter over each 4-core group sums the proj partials;
core rank r in the group returns output rows [r*1024, (r+1)*1024).

Device-side dataflow (all matmuls bf16 with f32 PSUM accumulation):
  - x [4096,768] is PE-transposed (via identity matmul) into xT tiles, bf16.
  - qT/kT computed d-major ([head-dim, T]) with heads packed 2-per-128
    partitions so S^T matmuls for head pairs row-tile concurrently.
  - v computed T-major in 65-column groups per 128-token block: cols 0-63 = v,
    col 64 = ones, so the PV matmul also produces the softmax denominator.
  - Attention per q-chunk of 512: S^T[k,q] = kT.T @ qT, P = exp(S/8) (bf16),
    diagonal blocks masked by 4 static causal masks, y^T accumulated in PSUM
    via lhsT=v_ext. No max-subtraction: |S| <= ~10 here so exp is f32-safe.
  - y^T normalized by the broadcast reciprocal of the denominator row
    (PE outer-product broadcast), written bf16.
  - proj: out-partial[tblock] = sum_h yT_h.T @ w_proj_rows_h, f32 to DRAM.
"""

import numpy as np
import ml_dtypes

B, T, C = 2, 4096, 768
H, HD = 12, 64
NCORES = 8
GROUPS = [[0, 1, 2, 3], [4, 5, 6, 7]]
HPC = 3           # heads per core
QC = 512          # q-chunk (and T-chunk) size
NQC = T // QC     # 8
KB = 128          # k-block size
NCB = C // 128    # 6 contraction blocks
NTB = T // 128    # 32 token blocks
VG = 65           # v-group width (64 v cols + 1 ones col)

BF16 = ml_dtypes.bfloat16

_CACHE = {}


def _build_program(use_rs=True):
    import concourse.bass as bass  # noqa: F401  (registers engines)
    import concourse.tile as tile
    from concourse import bacc, mybir

    DT = mybir.dt
    F32 = DT.float32
    BF = DT.bfloat16
    ADD = mybir.AluOpType.add
    EXP = mybir.ActivationFunctionType.Exp

    nc = bacc.Bacc("TRN2", target_bir_lowering=False, debug=False,
                   num_devices=NCORES)

    x_d = nc.dram_tensor("x", [T, C], F32, kind="ExternalInput")
    wqk_d = nc.dram_tensor("wqk", [C, 384], F32, kind="ExternalInput")
    wv_d = nc.dram_tensor("wv", [C, 192], F32, kind="ExternalInput")
    bqk_d = nc.dram_tensor("bqk", [128, 4], F32, kind="ExternalInput")
    bv_d = nc.dram_tensor("bv", [1, 192], BF, kind="ExternalInput")
    wp_d = nc.dram_tensor("wp", [192, C], F32, kind="ExternalInput")
    cm_d = nc.dram_tensor("cmask", [4, 128, QC], BF, kind="ExternalInput")
    id_d = nc.dram_tensor("ident", [128, 128], BF, kind="ExternalInput")
    if use_rs:
        out_d = nc.dram_tensor("out", [T // 4, C], F32, kind="ExternalOutput")
        part_d = nc.dram_tensor("part", [T, C], F32)
        rs_d = nc.dram_tensor("rs", [T // 4, C], F32)
    else:
        part_d = nc.dram_tensor("part", [T, C], F32, kind="ExternalOutput")

    from contextlib import ExitStack
    with tile.TileContext(nc) as tc, ExitStack() as es:
        pers = es.enter_context(tc.tile_pool(name="pers", bufs=1))
        stp = es.enter_context(tc.tile_pool(name="stage", bufs=3))
        xinp = es.enter_context(tc.tile_pool(name="xin", bufs=3))
        xbfp = es.enter_context(tc.tile_pool(name="xbf", bufs=3))
        xtp = es.enter_context(tc.tile_pool(name="xt", bufs=2))
        ppp = es.enter_context(tc.tile_pool(name="pp", bufs=2))
        psmm = es.enter_context(tc.tile_pool(name="psmm", bufs=2, space="PSUM"))
        pss = es.enter_context(tc.tile_pool(name="pss", bufs=1, space="PSUM"))
        psy = es.enter_context(tc.tile_pool(name="psy", bufs=1, space="PSUM"))

        # ---- persistent tiles ----
        ident = pers.tile([128, 128], BF, tag="ident")
        nc.sync.dma_start(out=ident[:], in_=id_d[:])
        cms = []
        for o in range(4):
            cm = pers.tile([128, QC], BF, tag=f"cm{o}")
            nc.sync.dma_start(out=cm[:], in_=cm_d[o, :, :])
            cms.append(cm)
        bqk = pers.tile([128, 4], F32, tag="bqk")
        nc.sync.dma_start(out=bqk[:], in_=bqk_d[:])
        bv = pers.tile([1, 192], BF, tag="bv")
        nc.sync.dma_start(out=bv[:], in_=bv_d[:])
        ones1 = pers.tile([1, 128], BF, tag="ones1")
        nc.vector.memset(ones1[:], 1.0)
        ones64 = pers.tile([1, 64], F32, tag="ones64")
        nc.vector.memset(ones64[:], 1.0)

        wqk_sb, wv_sb = [], []
        for cb in range(NCB):
            st = stp.tile([128, C], F32, tag="wst")
            nc.sync.dma_start(out=st[:, 0:384], in_=wqk_d[cb * 128:(cb + 1) * 128, :])
            w1 = pers.tile([128, 384], BF, tag=f"wqk{cb}")
            nc.vector.tensor_copy(w1[:], st[:, 0:384])
            wqk_sb.append(w1)
            st2 = stp.tile([128, C], F32, tag="wst")
            nc.sync.dma_start(out=st2[:, 0:192], in_=wv_d[cb * 128:(cb + 1) * 128, :])
            w2 = pers.tile([128, 192], BF, tag=f"wv{cb}")
            nc.vector.tensor_copy(w2[:], st2[:, 0:192])
            wv_sb.append(w2)
        st = stp.tile([128, C], F32, tag="wst")
        nc.sync.dma_start(out=st[:], in_=wp_d[0:128, :])
        wp01 = pers.tile([128, C], BF, tag="wp01")
        nc.vector.tensor_copy(wp01[:], st[:])
        st = stp.tile([128, C], F32, tag="wst")
        nc.sync.dma_start(out=st[0:64, :], in_=wp_d[128:192, :])
        wp2 = pers.tile([64, C], BF, tag="wp2")
        nc.vector.tensor_copy(wp2[:], st[0:64, :])

        q01 = pers.tile([128, T], BF, tag="q01")
        k01 = pers.tile([128, T], BF, tag="k01")
        q2 = pers.tile([64, T], BF, tag="q2")
        k2 = pers.tile([64, T], BF, tag="k2")
        y01 = pers.tile([128, T], BF, tag="y01")
        y2 = pers.tile([64, T], BF, tag="y2")
        vext = []
        for h in range(HPC):
            v = pers.tile([128, NTB * VG], BF, tag=f"vext{h}")
            # ones into column 64 of each 65-wide group
            nc.vector.memset(
                v[:].rearrange("p (g e) -> p g e", e=VG)[:, :, 64:65], 1.0)
            vext.append(v)

        # ---- main chunk loop ----
        for tcn in range(NQC):
            qs = slice(tcn * QC, (tcn + 1) * QC)
            # A) load x, cast, transpose into xT tiles for this chunk
            xts = [xtp.tile([128, QC], BF, tag=f"xt{cb}", name=f"xt{cb}") for cb in range(NCB)]
            for tb in range(4):
                tbg = tcn * 4 + tb
                xin = xinp.tile([128, C], F32, tag="xin")
                nc.sync.dma_start(out=xin[:], in_=x_d[tbg * 128:(tbg + 1) * 128, :])
                xbf = xbfp.tile([128, C], BF, tag="xbf")
                nc.vector.tensor_copy(xbf[:], xin[:])
                for cb in range(NCB):
                    pst = psmm.tile([128, 128], BF, tag="mm")
                    nc.tensor.transpose(pst[:], xbf[:, cb * 128:(cb + 1) * 128],
                                        ident[:])
                    nc.vector.tensor_copy(xts[cb][:, tb * 128:(tb + 1) * 128],
                                          pst[:])
            # B) v matmuls (T-major) + bias rank-1, write v_ext groups
            for tb in range(4):
                tbg = tcn * 4 + tb
                psv = psmm.tile([128, 192], F32, tag="mm")
                for cb in range(NCB):
                    nc.tensor.matmul(psv[:], lhsT=xts[cb][:, tb * 128:(tb + 1) * 128],
                                     rhs=wv_sb[cb][:], start=(cb == 0), stop=False)
                nc.tensor.matmul(psv[:], lhsT=ones1[:], rhs=bv[:],
                                 start=False, stop=True)
                for h in range(HPC):
                    nc.vector.tensor_copy(
                        vext[h][:, tbg * VG:tbg * VG + 64],
                        psv[:, h * 64:(h + 1) * 64])
            # C) q/k matmuls (d-major, head-pair packed)
            for mi, (c0, m, dest) in enumerate(
                    [(0, 128, q01), (128, 128, k01), (256, 64, q2), (320, 64, k2)]):
                psq = psmm.tile([128, QC], F32, tag="mm")
                for cb in range(NCB):
                    nc.tensor.matmul(psq[0:m, :], lhsT=wqk_sb[cb][:, c0:c0 + m],
                                     rhs=xts[cb][:], start=(cb == 0),
                                     stop=(cb == NCB - 1))
                nc.vector.tensor_scalar_add(dest[0:m, qs], psq[0:m, :],
                                            bqk[0:m, mi:mi + 1])
            # D) attention for q-chunk tcn
            nkb = 4 * tcn + 4
            psys = [psy.tile([VG, QC], F32, tag=f"y{h}", name=f"psy{h}") for h in range(HPC)]
            for kb in range(nkb):
                ks = slice(kb * 128, (kb + 1) * 128)
                srcs = [(k01, q01, 0), (k01, q01, 64), (k2, q2, 0)]
                for h in range(HPC):
                    kt, qt, p0 = srcs[h]
                    ps = pss.tile([128, QC], F32, tag=f"s{h}")
                    nc.tensor.matmul(ps[:], lhsT=kt[p0:p0 + 64, ks],
                                     rhs=qt[p0:p0 + 64, qs], start=True, stop=True)
                    pt = ppp.tile([128, QC], BF, tag=f"p{h}")
                    nc.scalar.activation(pt[:], ps[:], EXP, scale=0.125)
                    off = kb - 4 * tcn
                    if off >= 0:
                        nc.vector.tensor_mul(pt[:], pt[:], cms[off][:])
                    nc.tensor.matmul(psys[h][:], lhsT=vext[h][:, kb * VG:(kb + 1) * VG],
                                     rhs=pt[:], start=(kb == 0), stop=(kb == nkb - 1))
            # normalize: y = y_unnorm * broadcast(1/denom)
            for h, (ydest, p0) in enumerate([(y01, 0), (y01, 64), (y2, 0)]):
                recip = stp.tile([1, QC], F32, tag="recip")
                nc.vector.reciprocal(recip[:], psys[h][64:65, :])
                psrb = pss.tile([64, QC], F32, tag="s0")
                nc.tensor.matmul(psrb[:], lhsT=ones64[:], rhs=recip[:],
                                 start=True, stop=True)
                rb = stp.tile([64, QC], F32, tag="rb")
                nc.vector.tensor_copy(rb[:], psrb[:])
                nc.vector.tensor_mul(ydest[p0:p0 + 64, qs], psys[h][0:64, :], rb[:])
            # E) proj partials for this chunk's token blocks
            for tb in range(4):
                tbg = tcn * 4 + tb
                ts_ = slice(tbg * 128, (tbg + 1) * 128)
                stg = stp.tile([128, C], F32, tag="stg")
                for n0, nsz in [(0, 512), (512, 256)]:
                    psp = psmm.tile([128, 512], F32, tag="mm")
                    # NOTE: accumulating matmuls with different lhsT partition
                    # bases into one PSUM tile crash HW; keep all at base 0.
                    nc.tensor.matmul(psp[:, 0:nsz], lhsT=y01[:, ts_],
                                     rhs=wp01[:, n0:n0 + nsz],
                                     start=True, stop=False)
                    nc.tensor.matmul(psp[:, 0:nsz], lhsT=y2[0:64, ts_],
                                     rhs=wp2[0:64, n0:n0 + nsz],
                                     start=False, stop=True)
                    nc.vector.tensor_copy(stg[:, n0:n0 + nsz], psp[:, 0:nsz])
                nc.sync.dma_start(out=part_d[ts_, :], in_=stg[:])

        # F) reduce-scatter partials across the 4-core group, emit slice
        if use_rs:
            nc.gpsimd.collective_compute(
                "ReduceScatter", ADD, replica_groups=GROUPS,
                ins=[part_d[:]], outs=[rs_d[:]])
            nc.sync.dma_start(out=out_d[:], in_=rs_d[:])

    nc.compile()
    return nc


def _make_core_inputs(x, w_qkv, b_qkv, w_proj, core):
    b, g = core // 4, core % 4
    h0 = HPC * g
    wq = [w_qkv[:, (h0 + i) * HD:(h0 + i + 1) * HD] for i in range(HPC)]
    wk = [w_qkv[:, C + (h0 + i) * HD:C + (h0 + i + 1) * HD] for i in range(HPC)]
    wqk = np.concatenate([wq[0], wq[1], wk[0], wk[1], wq[2], wk[2]], axis=1)
    wv = w_qkv[:, 2 * C + h0 * HD:2 * C + (h0 + HPC) * HD]
    bq = [b_qkv[(h0 + i) * HD:(h0 + i + 1) * HD] for i in range(HPC)]
    bk = [b_qkv[C + (h0 + i) * HD:C + (h0 + i + 1) * HD] for i in range(HPC)]
    z = np.zeros(HD, np.float32)
    bqk = np.stack([
        np.concatenate([bq[0], bq[1]]), np.concatenate([bk[0], bk[1]]),
        np.concatenate([bq[2], z]), np.concatenate([bk[2], z])], axis=1)
    bv = b_qkv[2 * C + h0 * HD:2 * C + (h0 + HPC) * HD][None, :]
    wp = w_proj[h0 * HD:(h0 + HPC) * HD, :]
    return {
        "x": np.ascontiguousarray(x[b], np.float32),
        "wqk": np.ascontiguousarray(wqk, np.float32),
        "wv": np.ascontiguousarray(wv, np.float32),
        "bqk": np.ascontiguousarray(bqk, np.float32),
        "bv": np.ascontiguousarray(bv).astype(BF16),
        "wp": np.ascontiguousarray(wp, np.float32),
        "cmask": _causal_masks(),
        "ident": np.eye(128, dtype=np.float32).astype(BF16),
    }


def _causal_masks():
    k = np.arange(128)[:, None]
    q = np.arange(QC)[None, :]
    return np.stack([(k + o * 128 <= q) for o in range(4)]).astype(BF16)


def make_in_maps(x, w_qkv, b_qkv, w_proj):
    x = np.asarray(x, np.float32)
    w_qkv = np.asarray(w_qkv, np.float32)
    b_qkv = np.asarray(b_qkv, np.float32)
    w_proj = np.asarray(w_proj, np.float32)
    return [_make_core_inputs(x, w_qkv, b_qkv, w_proj, c) for c in range(NCORES)]


USE_RS = True


def get_program():
    key = ("nc", USE_RS)
    if key not in _CACHE:
        _CACHE[key] = _build_program(USE_RS)
    return _CACHE[key]


def assemble_output(results, b_proj):
    b_proj = np.asarray(b_proj, np.float32)
    out = np.empty((B, T, C), np.float32)
    if "out" in results[0]:
        for b in range(B):
            for r in range(4):
                out[b, r * 1024:(r + 1) * 1024] = results[4 * b + r]["out"]
    else:
        for b in range(B):
            out[b] = sum(results[4 * b + r]["part"] for r in range(4))
    out += b_proj
    return out


def kernel(x, w_qkv, b_qkv, w_proj, b_proj):
    from concourse.bass_utils import run_bass_kernel_spmd
    nc = get_program()
    in_maps = make_in_maps(x, w_qkv, b_qkv, w_proj)
    res = run_bass_kernel_spmd(nc, in_maps, list(range(NCORES)))
    return assemble_output(res.results, b_proj)
